# revision 17
# baseline (speedup 1.0000x reference)
"""Trainium2 Bass kernel for nn_DensityDecoder (gnn_message_passing).

Math: the reference computes, for every ordered pair (i, j) of NB=640 orbitals,
    pair = orb_i + orb_j                       (orb: per-orbital projected embedding)
    qn   = LayerNorm(pair) ; q = qn @ Wq + bq
    attn = softmax(q . k / sqrt(Dh)) over a tiny T=32 latent KV
    out  = MLP(attn @ V @ Wo)  ->  2 values -> rho[i, j] = out0 + 1j*out1

Because pair = orb_i + orb_j, the LN statistics decompose exactly:
    mu_ij  = mu_i + mu_j
    var_ij = msq_i + msq_j + 2*G_ij - mu_ij^2        (G = orb @ orb.T / D)
and the whole pre-softmax pipeline collapses to per-orbital precomputes
(SA = ((orb*g) @ Wq) projected into (head, token) score space, plus the
constant vectors Sw, Sb), so the 410MB pair tensor is never materialized:
    scores_ij = rstd_ij * (SA_i + SA_j - mu_ij*Sw + invr_ij*Sb)
The per-pair device work is the softmax + a 5-layer MLP chain, where
attn @ V @ Wo is folded into one 256x256 matmul (Wvo = blockdiag(V) @ Wo).

rho is exactly symmetric (pair_ij == pair_ji bitwise), so only j-blocks >=
i-block are computed (240 of 400 row-tiles) and the lower triangle is mirrored.

Sharding: rows i are striped across the 8 cores (i % 8 == core), giving every
core an IDENTICAL instruction stream (same NEFF, SPMD) over different data:
80 rows -> 240 tiles of 128 pairs -> 60 chain-chunks of 512 pairs.

Perf design (v2): everything on-device streams in fp16 (PE fp32r runs at
quarter rate below 512-wide moving dims; fp16 is full rate at any width, and
2-byte operands unlock the DVE 2x read mode). Scores for two tiles accumulate
into one PSUM bank, the rstd scale is applied by a DVE multiply (broadcast
along the 256 score columns), and the exp runs as ONE [128,1024] activation
per 512-pair chunk instead of four [128,256] ones. Scores are in [-4.4, 4.2]
on this input distribution, so fp16 exp/den never overflow. The attn
transposes are fp16 (1.0 PE cycles/row vs 1.5 for f32r) and their PSUM->SBUF
drain uses the DVE 2x mode.
"""

import os
import numpy as np

EPS = 1e-5
H = 8
D = 256
T = 32
Dh = D // H
NB = 640
NCORES = 8
NBLK = NB // 128          # 5 column blocks
RPB = 128 // NCORES       # 16 rows per block per core
NROWS = NBLK * RPB        # 80 rows per core
# tile enumeration (identical on every core): (block, row-in-block, j-block)
TILES = [(B, r, jt) for B in range(NBLK) for r in range(RPB) for jt in range(B, NBLK)]
NTILES = len(TILES)       # 240
CHUNK = 4                 # tiles per MLP-chain chunk (512 pair columns)
GROUP = int(os.environ.get("DD_GROUP", "8"))  # chunks per superchunk (ACT-phase granularity)
NCHUNKS = NTILES // CHUNK  # 60

_CACHE = {}


def _silu(x):
    return x / (1.0 + np.exp(-x))


def _ln(x, g, b):
    mu = x.mean(-1, keepdims=True)
    var = x.var(-1, keepdims=True)
    return (x - mu) / np.sqrt(var + EPS) * g + b


def _precompute(inputs):
    """Pair-independent precompute (all O(NB*D) or smaller), numpy float64 -> float32."""
    f = {}
    for k, v in inputs.items():
        v = np.asarray(v)
        f[k] = v.astype(np.float64) if v.dtype in (np.float32, np.float64) else v
    Z = np.asarray(inputs["Z"]).astype(np.int64)
    l = np.asarray(inputs["l"]).astype(np.int64)
    m = np.asarray(inputs["m"]).astype(np.int64)
    m_idx = np.clip(m + 3, 0, 4)
    emb = np.concatenate([f["elem_tab"][Z], f["l_tab"][l], f["m_tab"][m_idx]], -1)
    orb = _silu(emb @ f["Wp0"] + f["bp0"]) @ f["Wp1"] + f["bp1"]          # (NB, D)

    kv = _ln(f["latent"], f["ln_gkv"], f["ln_bkv"])
    k = (kv @ f["Wk"] + f["bk"]).reshape(T, H, Dh)
    v = (kv @ f["Wv"] + f["bv"]).reshape(T, H, Dh)

    g, b = f["ln_gq"], f["ln_bq"]
    mu = orb.mean(-1)
    msq = (orb * orb).mean(-1)

    A = (orb * g) @ f["Wq"]
    wbar = g @ f["Wq"]
    bq_eff = b @ f["Wq"] + f["bqa"]

    kT = k.transpose(1, 2, 0)                                            # (H, Dh, T)
    scale = 1.0 / np.sqrt(np.float64(Dh))

    def to_scores(x):
        xh = x.reshape(x.shape[:-1] + (H, Dh))
        return (np.einsum('...hd,hdt->...ht', xh, kT).reshape(x.shape[:-1] + (H * T,))
                * scale)

    SA = to_scores(A)                                                    # (NB, 256)
    Sw = to_scores(wbar)                                                 # (256,)
    Sb = to_scores(bq_eff)                                               # (256,)
    Wvo = np.einsum('thd,hde->hte', v, f["Wo"].reshape(H, Dh, D)).reshape(H * T, D)
    # fuse consecutive linear layers (no nonlinearity between them):
    # y2 = silu(attn @ Wa + ba); y4 = silu(y2 @ Wb + bb); y5 = silu(y4 @ Wd1 + bd1)
    Wa = Wvo @ f["Wt0"]
    ba = f["bo"] @ f["Wt0"] + f["bt0"]
    Wb = f["Wt1"] @ f["Wd0"]
    bb = f["bt1"] @ f["Wd0"] + f["bd0"]

    fl = lambda x: np.ascontiguousarray(x, np.float32)
    fh = lambda x: np.ascontiguousarray(x, np.float16)
    return {
        "SA": fh(SA), "Sw": fh(Sw), "Sb": fh(Sb), "mu": fl(mu), "msq": fl(msq),
        "orbT_s": fl(orb.T * np.sqrt(2.0 / D)),                          # (D, NB)
        "Wa": fh(Wa), "ba": fl(ba), "Wb": fh(Wb), "bb": fl(bb),
        "Wd1": fh(f["Wd1"]), "bd1": fl(f["bd1"]),
        "Wd2": fh(f["Wd2"]), "bd2": fl(f["bd2"]),
    }


def core_rows(c):
    return [B * 128 + r * NCORES + c for B in range(NBLK) for r in range(RPB)]


def _core_inputs(pc, c):
    rows = core_rows(c)
    ones80 = np.ones(NROWS, np.float32)
    return {
        "sa_in": pc["SA"],
        "orbT_in": pc["orbT_s"],
        "orbTc_in": np.ascontiguousarray(pc["orbT_s"][:, rows]),
        "lhs_mu": np.ascontiguousarray(np.stack([ones80, pc["mu"][rows]])),
        "lhs_msq": np.ascontiguousarray(np.stack([ones80, pc["msq"][rows]])),
        "rhs_mu": np.ascontiguousarray(np.stack([pc["mu"], np.ones(NB, np.float32)])),
        "rhs_msq": np.ascontiguousarray(np.stack([pc["msq"], np.ones(NB, np.float32)])),
        "r3_all": np.ascontiguousarray(np.stack(
            [np.stack([pc["SA"][i], -pc["Sw"], pc["Sb"]]) for i in rows])),
        "ident_in": np.eye(128, dtype=np.float16),
        "ones_in": np.ones((1, NB), np.float16),
        "wa": pc["Wa"], "wb": pc["Wb"], "wd1": pc["Wd1"], "wd2": pc["Wd2"],
        "ba_in": pc["ba"], "bb_in": pc["bb"],
        "bd1_in": pc["bd1"], "bd2_in": pc["bd2"],
    }


def _build_nc(n_chunks):
    import concourse.bass as bass
    import concourse.bacc as bacc
    import concourse.tile as tile
    from concourse import mybir
    dt = mybir.dt
    f32 = dt.float32
    f32r = dt.float32r
    f16 = dt.float16
    AF = mybir.ActivationFunctionType
    AX = mybir.AxisListType

    nc = bacc.Bacc(None, target_bir_lowering=False)

    ein = lambda name, shape, d=f32: nc.dram_tensor(name, shape, d,
                                                     kind="ExternalInput")
    sa_in = ein("sa_in", [NB, 256], f16)
    orbT_in = ein("orbT_in", [D, NB], f32r)
    orbTc_in = ein("orbTc_in", [D, NROWS], f32r)
    lhs_mu = ein("lhs_mu", [2, NROWS], f32r)
    lhs_msq = ein("lhs_msq", [2, NROWS], f32r)
    rhs_mu = ein("rhs_mu", [2, NB], f32r)
    rhs_msq = ein("rhs_msq", [2, NB], f32r)
    r3_all = ein("r3_all", [NROWS, 3, 256], f16)
    ident_in = ein("ident_in", [128, 128], f16)
    ones_in = ein("ones_in", [1, NB], f16)
    wa = ein("wa", [256, 256], f16)
    wb = ein("wb", [256, 256], f16)
    wd1 = ein("wd1", [256, 256], f16)
    wd2 = ein("wd2", [256, 2], f16)
    ba_in = ein("ba_in", [256])
    bb_in = ein("bb_in", [256])
    bd1_in = ein("bd1_in", [256])
    bd2_in = ein("bd2_in", [2])

    out_ext = nc.dram_tensor("out", [NCHUNKS, 2, 512], f32, kind="ExternalOutput")
    stats_dram = nc.dram_tensor("stats_scratch", [2, NROWS, NB], f16)

    # start offsets of each B-block in the TILES enumeration
    tstart = [0] * (NBLK + 1)
    for B in range(NBLK):
        tstart[B + 1] = tstart[B] + RPB * (NBLK - B)

    with tile.TileContext(nc) as tc, \
            nc.allow_low_precision(reason="fp16 pipeline by design"):
        with (
            tc.tile_pool(name="const", bufs=1) as const,
            tc.tile_pool(name="prow", bufs=2) as prow,
            tc.tile_pool(name="score", bufs=int(os.environ.get("DD_SCORE", str(GROUP + 1)))) as score,
            tc.tile_pool(name="small", bufs=5) as small,
            tc.tile_pool(name="attnT", bufs=2 * GROUP + 3) as attnT_pool,
            tc.tile_pool(name="chainx", bufs=int(os.environ.get("DD_CHX", "2"))) as chainx,
        ):
            # ---- constants into SBUF ----
            sa = const.tile([128, NBLK, 256], f16)
            nc.sync.dma_start(out=sa, in_=sa_in.rearrange("(jt p) c -> p jt c", p=128))
            orbT = const.tile([128, 2, NB], f32r)
            nc.sync.dma_start(out=orbT, in_=orbT_in.rearrange("(k p) n -> p k n", p=128))
            orbTc = const.tile([128, 2, NROWS], f32r)
            nc.sync.dma_start(out=orbTc, in_=orbTc_in.rearrange("(k p) m -> p k m", p=128))
            lmu = const.tile([2, NROWS], f32r)
            nc.sync.dma_start(out=lmu, in_=lhs_mu[:])
            lmsq = const.tile([2, NROWS], f32r)
            nc.sync.dma_start(out=lmsq, in_=lhs_msq[:])
            rmu = const.tile([2, NB], f32r)
            nc.sync.dma_start(out=rmu, in_=rhs_mu[:])
            rmsq = const.tile([2, NB], f32r)
            nc.sync.dma_start(out=rmsq, in_=rhs_msq[:])

            w_a = const.tile([128, 2, 256], f16)
            nc.sync.dma_start(out=w_a, in_=wa.rearrange("(k p) n -> p k n", p=128))
            w_b = const.tile([128, 2, 256], f16)
            nc.sync.dma_start(out=w_b, in_=wb.rearrange("(k p) n -> p k n", p=128))
            w_d1 = const.tile([128, 2, 256], f16)
            nc.sync.dma_start(out=w_d1, in_=wd1.rearrange("(k p) n -> p k n", p=128))
            w_d2 = const.tile([128, 2, 2], f16)
            nc.sync.dma_start(out=w_d2, in_=wd2.rearrange("(k p) n -> p k n", p=128))

            b_a = const.tile([128, 2], f32)
            nc.sync.dma_start(out=b_a, in_=ba_in.rearrange("(m p) -> p m", p=128))
            b_b = const.tile([128, 2], f32)
            nc.sync.dma_start(out=b_b, in_=bb_in.rearrange("(m p) -> p m", p=128))
            b_d1 = const.tile([128, 2], f32)
            nc.sync.dma_start(out=b_d1, in_=bd1_in.rearrange("(m p) -> p m", p=128))
            b_d2 = const.tile([2, 1], f32)
            nc.sync.dma_start(out=b_d2, in_=bd2_in.rearrange("(p one) -> p one", one=1))

            ident = const.tile([128, 128], f16)
            nc.sync.dma_start(out=ident, in_=ident_in[:])
            l3_bufs = [const.tile([3, NB], f16, tag=f"l3_{i}", name=f"l3_{i}")
                       for i in range(2)]
            for lb in l3_bufs:
                nc.sync.dma_start(out=lb[0:1, :], in_=ones_in[:])
            eps_t = const.tile([NROWS, 1], f32)
            nc.gpsimd.memset(eps_t, EPS)

            # persistent per-row stats
            mu_p_sb = const.tile([NROWS, NB], f32r)
            invr_sb = const.tile([NROWS, NB], f32r)
            rstd_sb = const.tile([NROWS, NB], f16)
            rstd_T = const.tile([128, NBLK, NROWS], f16)
            rstdq = const.tile([128, NTILES], f16)

            # ---- prologue: per-pair LN stats for this core's 80 rows ----
            with (
                tc.tile_pool(name="pro_ps", bufs=2, space="PSUM") as pro_ps,
                tc.tile_pool(name="pro_sb", bufs=2) as pro_sb,
            ):
                for nch in range(2):
                    seg = slice(nch * 320, (nch + 1) * 320)
                    psA = pro_ps.tile([NROWS, 320], f32, tag="psA")
                    nc.tensor.matmul(psA, lmu, rmu[:, seg], start=True, stop=True)
                    nc.vector.tensor_copy(out=mu_p_sb[:, seg], in_=psA)
                    psB = pro_ps.tile([NROWS, 320], f32, tag="psB")
                    nc.tensor.matmul(psB, lmsq, rmsq[:, seg], start=True, stop=False)
                    nc.tensor.matmul(psB, orbTc[:, 0, :], orbT[:, 0, seg],
                                     start=False, stop=False)
                    nc.tensor.matmul(psB, orbTc[:, 1, :], orbT[:, 1, seg],
                                     start=False, stop=True)
                    mu2 = pro_sb.tile([NROWS, 320], f32, tag="mu2")
                    nc.vector.tensor_mul(mu2, mu_p_sb[:, seg], mu_p_sb[:, seg])
                    nc.vector.tensor_sub(invr_sb[:, seg], psB, mu2)
                # invr = sqrt(var + eps); rstd = 1/invr
                nc.scalar.activation(out=invr_sb, in_=invr_sb, func=AF.Sqrt,
                                     bias=eps_t[:, 0:1])
                nc.vector.reciprocal(out=rstd_sb, in_=invr_sb)
                # fp16 stats to DRAM (fetched per row into l3 lhsT buffers)
                m16 = pro_sb.tile([NROWS, NB], f16, tag="m16")
                nc.vector.tensor_copy(out=m16, in_=mu_p_sb)
                i16 = pro_sb.tile([NROWS, NB], f16, tag="i16")
                nc.vector.tensor_copy(out=i16, in_=invr_sb)
                nc.sync.dma_start(out=stats_dram[0], in_=m16)
                nc.sync.dma_start(out=stats_dram[1], in_=i16)
                for jt in range(NBLK):
                    pT = pro_ps.tile([128, NROWS], f16, tag="pT")
                    nc.tensor.transpose(
                        pT, rstd_sb[:, jt * 128:(jt + 1) * 128],
                        ident[0:NROWS, 0:NROWS])
                    nc.vector.tensor_copy(out=rstd_T[:, jt, :], in_=pT)
                # rstd per (tile, j-lane) in TILES order: for block B the
                # tiles run (r, jt) row-major, jt in [B, NBLK). DVE copies
                # (a strided DMA here shatters into 31K 2-byte descriptors).
                for B in range(NBLK):
                    for jt in range(B, NBLK):
                        nc.vector.tensor_copy(
                            out=rstdq[:, tstart[B]:tstart[B + 1]]
                                .rearrange("p (r jt) -> p r jt", jt=NBLK - B)
                                [:, :, jt - B],
                            in_=rstd_T[:, jt, B * RPB:(B + 1) * RPB])

            # ---- main loop: superchunks separate Exp (phase A) from Silu
            # (phase B) on the scalar engine. Each activation-function switch
            # costs a ~1.3us InstLoadActFuncSet table load, so ACT program
            # order is pinned with order-only deps: [A exps][B silus] per
            # superchunk -> 2 table loads per superchunk instead of per tile.
            # Other engines still overlap phase B(s) with phase A(s+1).
            from concourse.tile_rust import add_dep_helper
            import contextlib
            _mstack = contextlib.ExitStack()
            px3_pool = _mstack.enter_context(
                tc.tile_pool(name="px3", bufs=int(os.environ.get("DD_PX3", "2")), space="PSUM"))
            ptr_pool = _mstack.enter_context(
                tc.tile_pool(name="ptr", bufs=int(os.environ.get("DD_PTR", "2")), space="PSUM"))
            pchain = _mstack.enter_context(
                tc.tile_pool(name="pchain", bufs=int(os.environ.get("DD_PCH", "2")), space="PSUM"))
            attnn_pool = _mstack.enter_context(
                tc.tile_pool(name="attnn", bufs=2 * GROUP + 2))
            repeat = int(os.environ.get("DD_REPEAT", "1"))

            act_prev = [None]
            nopin = bool(int(os.environ.get("DD_NOPIN", "0")))

            def act_chain(bi):
                if act_prev[0] is not None and not nopin:
                    add_dep_helper(bi.ins, act_prev[0].ins, sync=True,
                                   reason="pin ACT order for act-table reuse")
                act_prev[0] = bi
                return bi

            prev_row = [None, None, None]   # r_loc, l3row, r3

            def score_half(t0, scored, hi):
                """Scores for tiles (t0, t0+1) -> scored[:, 2*hi:2*hi+2, :]
                (fp16, pre-scaled by rstd). One PSUM bank for both tiles.
                PSUM start=True zeroes a whole 2KB bank (ZERO_REGION), so the
                bank's FIRST matmul must be the only one with start=True; the
                SA_j ident pass goes first and the rank-3s accumulate onto it.
                """
                px = px3_pool.tile([128, 2, 256], f32, tag="px3", name="px3")
                rows = []
                jts = []
                for ti in range(2):
                    t = t0 + ti
                    B, r, jt = TILES[t]
                    jts.append(jt)
                    r_loc = B * RPB + r
                    if prev_row[0] != r_loc:
                        prev_row[0] = r_loc
                        # lhsT rows: [ones; mu_p(row); invr(row)];
                        # rhs rows: [SA_i; -Sw; Sb]
                        l3row = l3_bufs[r_loc % 2]
                        nc.sync.dma_start(out=l3row[1:3, :],
                                          in_=stats_dram[:, r_loc, :])
                        r3 = prow.tile([3, 256], f16, tag="r3", name="r3")
                        nc.sync.dma_start(out=r3, in_=r3_all[r_loc])
                        prev_row[1], prev_row[2] = l3row, r3
                    rows.append((prev_row[1], prev_row[2], jt))
                # SA_j (broadcast add via identity matmul) first: one N=512
                # pass when the two j-blocks are adjacent in the sa tile
                if jts[1] == jts[0] + 1:
                    nc.tensor.matmul(px.rearrange("p a c -> p (a c)"), ident,
                                     sa[:, jts[0]:jts[0] + 2, :]
                                         .rearrange("p a c -> p (a c)"),
                                     start=True, stop=False,
                                     skip_group_check=True)
                else:
                    for ti in range(2):
                        nc.tensor.matmul(px[:, ti, :], ident, sa[:, jts[ti], :],
                                         start=(ti == 0), stop=False,
                                         skip_group_check=True)
                # rank-3: + SA_i - mu*Sw + invr*Sb
                for ti in range(2):
                    l3row, r3, jt = rows[ti]
                    jseg = slice(jt * 128, (jt + 1) * 128)
                    nc.tensor.matmul(px[:, ti, :], l3row[:, jseg], r3,
                                     start=False, stop=(ti == 1),
                                     skip_group_check=True)
                # scores = rstd * px  (DVE; rstd broadcast along 256 columns)
                nc.vector.tensor_mul(
                    scored[:, 2 * hi:2 * hi + 2, :], px,
                    rstdq[:, t0:t0 + 2].to_broadcast([128, 2, 256]))

            def score_chunk(q):
                """Softmax for one 4-tile chunk -> normalized attn (fp16)."""
                scored = score.tile([128, 4, 256], f16, tag="sc", name="sc")
                for hi in range(2):
                    score_half(q * CHUNK + 2 * hi, scored, hi)
                ee = score.tile([128, 4, 8, 32], f16, tag="ee", name="ee")
                act_chain(nc.scalar.activation(
                    out=ee.rearrange("p a h t -> p (a h t)"),
                    in_=scored.rearrange("p a c -> p (a c)"),
                    func=AF.Exp))
                den = small.tile([128, 4, 8], f16, tag="den", name="den")
                nc.vector.reduce_sum(out=den, in_=ee, axis=AX.X)
                rden = small.tile([128, 4, 8], f16, tag="rden", name="rden")
                nc.vector.reciprocal(out=rden, in_=den)
                attn = attnn_pool.tile([128, 4, 8, 32], f16, tag="attn",
                                       name="attn")
                nc.gpsimd.tensor_mul(attn, ee,
                                     rden.to_broadcast([128, 4, 8, 32]))
                return attn

            def transpose_chunk(attn):
                """PE-transpose one chunk's attn into SBUF aT for the chain."""
                ptrt = ptr_pool.tile([128, 2, 512], f16, tag="ptrt",
                                     name="ptrt")
                for ti in range(CHUNK):
                    a2 = attn[:, ti, :, :].rearrange("p h t -> p (h t)")
                    sseg = slice(ti * 128, (ti + 1) * 128)
                    nc.tensor.transpose(ptrt[:, 0, sseg], a2[:, 0:128], ident)
                    nc.tensor.transpose(ptrt[:, 1, sseg], a2[:, 128:256], ident)
                aT = attnT_pool.tile([128, 2, 512], f16, tag="aT", name="aT")
                nc.vector.tensor_copy(out=aT, in_=ptrt)
                return aT

            def chain_pair(aT_pair, q_pair):
                # two chunks share each silu: psum [128, 2, 512] spans two
                # banks, one [128, 1024] activation per (layer, mt) halves
                # the scalar engine's fixed per-op cost.
                def layer(x_of, w, b_tile, out_tile):
                    for mt in range(2):
                        ps = pchain.tile([128, 2, 512], f32, tag="pch",
                                         name="pch")
                        for qi in range(2):
                            for kt in range(2):
                                nc.tensor.matmul(
                                    ps[:, qi, :],
                                    w[:, kt, mt * 128:(mt + 1) * 128],
                                    x_of(qi, kt),
                                    start=(kt == 0), stop=(kt == 1))
                        act_chain(nc.scalar.activation(
                            out=out_tile[:, mt, :, :].rearrange(
                                "p q n -> p (q n)"),
                            in_=ps.rearrange("p q n -> p (q n)"), func=AF.Silu,
                            bias=b_tile[:, mt:mt + 1]))

                x2 = chainx.tile([128, 2, 2, 512], f16, tag="x2", name="x2")
                layer(lambda qi, kt: aT_pair[qi][:, kt, :], w_a, b_a, x2)
                x4 = chainx.tile([128, 2, 2, 512], f16, tag="x4", name="x4")
                layer(lambda qi, kt: x2[:, kt, qi, :], w_b, b_b, x4)
                x5 = chainx.tile([128, 2, 2, 512], f16, tag="x5", name="x5")
                layer(lambda qi, kt: x4[:, kt, qi, :], w_d1, b_d1, x5)
                ps6 = pchain.tile([2, 2, 512], f32, tag="pch", name="ps6")
                for qi in range(2):
                    for kt in range(2):
                        nc.tensor.matmul(ps6[:, qi, :], w_d2[:, kt, :],
                                         x5[:, kt, qi, :],
                                         start=(kt == 0), stop=(kt == 1))
                # bias bd2 is added host-side during assembly
                o6 = small.tile([2, 2, 512], f32, tag="o6", name="o6")
                nc.vector.tensor_copy(out=o6, in_=ps6)
                for qi in range(2):
                    nc.sync.dma_start(out=out_ext[q_pair[qi]], in_=o6[:, qi, :])

            n_super = (n_chunks + GROUP - 1) // GROUP

            for rep in range(repeat):
                prev_row[0] = None
                # PE program order per superchunk: [scores sc][chains sc-1]
                # [transposes sc]. The chains give PE ~30us of work while the
                # DVE->ACT->DVE->gpsimd softmax pipeline for sc drains, so PE
                # never head-of-line blocks on a transpose whose attn isn't
                # ready. ACT order stays [exps sc][silus sc-1] (table reuse).
                def run_chains(p):
                    qs, aTs = p
                    for i in range(0, len(qs) - 1, 2):
                        chain_pair(aTs[i:i + 2], qs[i:i + 2])

                # Softmaxed attn tiles of superchunk s are transposed and
                # drained one superchunk later, interleaved chunk-by-chunk
                # with s+1's score work: PE sees [scores q][transposes q']...
                # with both operands long ready, and the DVE aT drains sit
                # between (not ahead of) the prescales they'd otherwise block.
                prev = None          # (qs, attns) of superchunk sc-1
                for sc in range(n_super + 1):
                    qs = (list(range(sc * GROUP, min((sc + 1) * GROUP, n_chunks)))
                          if sc < n_super else [])
                    attns = []
                    aTs = []
                    for idx, q in enumerate(qs):
                        attns.append(score_chunk(q))
                        if prev and idx < len(prev[1]):
                            aTs.append(transpose_chunk(prev[1][idx]))
                    if prev:
                        for a in prev[1][len(aTs):]:
                            aTs.append(transpose_chunk(a))
                        run_chains((prev[0], aTs))
                    prev = (qs, attns)
            _mstack.close()
    nc.compile()
    return nc


def _get_nc(n_chunks):
    key = ("nc", n_chunks)
    if key not in _CACHE:
        _CACHE[key] = _build_nc(n_chunks)
    return _CACHE[key]


def kernel(**inputs):
    from concourse.bass_utils import run_bass_kernel_spmd

    n_chunks = int(os.environ.get("DD_CHUNKS", NCHUNKS))
    pc = _precompute(inputs)
    in_maps = [_core_inputs(pc, c) for c in range(NCORES)]
    nc = _get_nc(n_chunks)
    res = run_bass_kernel_spmd(nc, in_maps, core_ids=list(range(NCORES)),
                               trace=bool(int(os.environ.get("DD_TRACE", "0"))))
    _CACHE["last_result"] = res

    R = np.zeros((NB, NB, 2), np.float32)
    for c in range(NCORES):
        o = res.results[c]["out"] + pc["bd2"][None, :, None]   # (NCHUNKS, 2, 512)
        ot = o.reshape(NCHUNKS, 2, CHUNK, 128).transpose(0, 2, 1, 3).reshape(-1, 2, 128)
        for t in range(n_chunks * CHUNK):
            B, r, jt = TILES[t]
            i = B * 128 + r * NCORES + c
            R[i, jt * 128:(jt + 1) * 128, 0] = ot[t, 0]
            R[i, jt * 128:(jt + 1) * 128, 1] = ot[t, 1]
    for bi in range(NBLK):
        for bj in range(bi):
            R[bi * 128:(bi + 1) * 128, bj * 128:(bj + 1) * 128] = \
                R[bj * 128:(bj + 1) * 128, bi * 128:(bi + 1) * 128].transpose(1, 0, 2)

    rho = (R[:, :, 0] + 1j * R[:, :, 1]).astype(np.complex64)
    n_spin = int(np.asarray(inputs["n_spin"]))
    return np.broadcast_to(rho[None], (n_spin, NB, NB)).copy()


# revision 22
# speedup vs baseline: 1.0968x; 1.0968x over previous
"""Trainium2 Bass kernel for nn_DensityDecoder (gnn_message_passing).

Math: the reference computes, for every ordered pair (i, j) of NB=640 orbitals,
    pair = orb_i + orb_j                       (orb: per-orbital projected embedding)
    qn   = LayerNorm(pair) ; q = qn @ Wq + bq
    attn = softmax(q . k / sqrt(Dh)) over a tiny T=32 latent KV
    out  = MLP(attn @ V @ Wo)  ->  2 values -> rho[i, j] = out0 + 1j*out1

Because pair = orb_i + orb_j, the LN statistics decompose exactly:
    mu_ij  = mu_i + mu_j
    var_ij = msq_i + msq_j + 2*G_ij - mu_ij^2        (G = orb @ orb.T / D)
and the whole pre-softmax pipeline collapses to per-orbital precomputes
(SA = ((orb*g) @ Wq) projected into (head, token) score space, plus the
constant vectors Sw, Sb), so the 410MB pair tensor is never materialized:
    scores_ij = rstd_ij * (SA_i + SA_j - mu_ij*Sw + invr_ij*Sb)
The per-pair device work is the softmax + a 5-layer MLP chain, where
attn @ V @ Wo is folded into one 256x256 matmul (Wvo = blockdiag(V) @ Wo).

rho is exactly symmetric (pair_ij == pair_ji bitwise), so only j-blocks >=
i-block are computed (240 of 400 row-tiles) and the lower triangle is mirrored.

Sharding: rows i are striped across the 8 cores (i % 8 == core), giving every
core an IDENTICAL instruction stream (same NEFF, SPMD) over different data:
80 rows -> 240 tiles of 128 pairs -> 60 chain-chunks of 512 pairs.

Perf design (v2): everything on-device streams in fp16 (PE fp32r runs at
quarter rate below 512-wide moving dims; fp16 is full rate at any width, and
2-byte operands unlock the DVE 2x read mode). Scores for two tiles accumulate
into one PSUM bank, the rstd scale is applied by a DVE multiply (broadcast
along the 256 score columns), and the exp runs as ONE [128,1024] activation
per 512-pair chunk instead of four [128,256] ones. Scores are in [-4.4, 4.2]
on this input distribution, so fp16 exp/den never overflow. The attn
transposes are fp16 (1.0 PE cycles/row vs 1.5 for f32r) and their PSUM->SBUF
drain uses the DVE 2x mode.
"""

import os
import numpy as np

EPS = 1e-5
H = 8
D = 256
T = 32
Dh = D // H
NB = 640
NCORES = 8
NBLK = NB // 128          # 5 column blocks
RPB = 128 // NCORES       # 16 rows per block per core
NROWS = NBLK * RPB        # 80 rows per core
# tile enumeration (identical on every core): (block, row-in-block, j-block)
TILES = [(B, r, jt) for B in range(NBLK) for r in range(RPB) for jt in range(B, NBLK)]
NTILES = len(TILES)       # 240
CHUNK = 4                 # tiles per MLP-chain chunk (512 pair columns)
GROUP = int(os.environ.get("DD_GROUP", "8"))  # chunks per superchunk (ACT-phase granularity)
NCHUNKS = NTILES // CHUNK  # 60

_CACHE = {}


def _silu(x):
    return x / (1.0 + np.exp(-x))


def _ln(x, g, b):
    mu = x.mean(-1, keepdims=True)
    var = x.var(-1, keepdims=True)
    return (x - mu) / np.sqrt(var + EPS) * g + b


def _precompute(inputs):
    """Pair-independent precompute (all O(NB*D) or smaller), numpy float64 -> float32."""
    f = {}
    for k, v in inputs.items():
        v = np.asarray(v)
        f[k] = v.astype(np.float64) if v.dtype in (np.float32, np.float64) else v
    Z = np.asarray(inputs["Z"]).astype(np.int64)
    l = np.asarray(inputs["l"]).astype(np.int64)
    m = np.asarray(inputs["m"]).astype(np.int64)
    m_idx = np.clip(m + 3, 0, 4)
    emb = np.concatenate([f["elem_tab"][Z], f["l_tab"][l], f["m_tab"][m_idx]], -1)
    orb = _silu(emb @ f["Wp0"] + f["bp0"]) @ f["Wp1"] + f["bp1"]          # (NB, D)

    kv = _ln(f["latent"], f["ln_gkv"], f["ln_bkv"])
    k = (kv @ f["Wk"] + f["bk"]).reshape(T, H, Dh)
    v = (kv @ f["Wv"] + f["bv"]).reshape(T, H, Dh)

    g, b = f["ln_gq"], f["ln_bq"]
    mu = orb.mean(-1)
    msq = (orb * orb).mean(-1)

    A = (orb * g) @ f["Wq"]
    wbar = g @ f["Wq"]
    bq_eff = b @ f["Wq"] + f["bqa"]

    kT = k.transpose(1, 2, 0)                                            # (H, Dh, T)
    scale = 1.0 / np.sqrt(np.float64(Dh))

    def to_scores(x):
        xh = x.reshape(x.shape[:-1] + (H, Dh))
        return (np.einsum('...hd,hdt->...ht', xh, kT).reshape(x.shape[:-1] + (H * T,))
                * scale)

    SA = to_scores(A)                                                    # (NB, 256)
    Sw = to_scores(wbar)                                                 # (256,)
    Sb = to_scores(bq_eff)                                               # (256,)
    Wvo = np.einsum('thd,hde->hte', v, f["Wo"].reshape(H, Dh, D)).reshape(H * T, D)
    # fuse consecutive linear layers (no nonlinearity between them):
    # y2 = silu(attn @ Wa + ba); y4 = silu(y2 @ Wb + bb); y5 = silu(y4 @ Wd1 + bd1)
    Wa = Wvo @ f["Wt0"]
    ba = f["bo"] @ f["Wt0"] + f["bt0"]
    Wb = f["Wt1"] @ f["Wd0"]
    bb = f["bt1"] @ f["Wd0"] + f["bd0"]

    fl = lambda x: np.ascontiguousarray(x, np.float32)
    fh = lambda x: np.ascontiguousarray(x, np.float16)
    return {
        "SA": fh(SA), "Sw": fh(Sw), "Sb": fh(Sb), "mu": fl(mu), "msq": fl(msq),
        "orbT_s": fl(orb.T * np.sqrt(2.0 / D)),                          # (D, NB)
        "Wa": fh(Wa), "ba": fl(ba), "Wb": fh(Wb), "bb": fl(bb),
        "Wd1": fh(f["Wd1"]), "bd1": fl(f["bd1"]),
        "Wd2": fh(f["Wd2"]), "bd2": fl(f["bd2"]),
    }


def core_rows(c):
    return [B * 128 + r * NCORES + c for B in range(NBLK) for r in range(RPB)]


def _core_inputs(pc, c):
    rows = core_rows(c)
    ones80 = np.ones(NROWS, np.float32)
    return {
        "sa_in": pc["SA"],
        "orbT_in": pc["orbT_s"],
        "orbTc_in": np.ascontiguousarray(pc["orbT_s"][:, rows]),
        "lhs_mu": np.ascontiguousarray(np.stack([ones80, pc["mu"][rows]])),
        "lhs_msq": np.ascontiguousarray(np.stack([ones80, pc["msq"][rows]])),
        "rhs_mu": np.ascontiguousarray(np.stack([pc["mu"], np.ones(NB, np.float32)])),
        "rhs_msq": np.ascontiguousarray(np.stack([pc["msq"], np.ones(NB, np.float32)])),
        "r3_all": np.ascontiguousarray(np.stack(
            [np.stack([pc["SA"][i], -pc["Sw"], pc["Sb"]]) for i in rows])),
        "ident_in": np.eye(128, dtype=np.float16),
        "ones_in": np.ones((1, NB), np.float16),
        "wa": pc["Wa"], "wb": pc["Wb"], "wd1": pc["Wd1"], "wd2": pc["Wd2"],
        "ba_in": pc["ba"], "bb_in": pc["bb"],
        "bd1_in": pc["bd1"], "bd2_in": pc["bd2"],
    }


def _build_nc(n_chunks):
    import concourse.bass as bass
    import concourse.bacc as bacc
    import concourse.tile as tile
    from concourse import mybir
    dt = mybir.dt
    f32 = dt.float32
    f32r = dt.float32r
    f16 = dt.float16
    AF = mybir.ActivationFunctionType
    AX = mybir.AxisListType

    nc = bacc.Bacc(None, target_bir_lowering=False)

    ein = lambda name, shape, d=f32: nc.dram_tensor(name, shape, d,
                                                     kind="ExternalInput")
    sa_in = ein("sa_in", [NB, 256], f16)
    orbT_in = ein("orbT_in", [D, NB], f32r)
    orbTc_in = ein("orbTc_in", [D, NROWS], f32r)
    lhs_mu = ein("lhs_mu", [2, NROWS], f32r)
    lhs_msq = ein("lhs_msq", [2, NROWS], f32r)
    rhs_mu = ein("rhs_mu", [2, NB], f32r)
    rhs_msq = ein("rhs_msq", [2, NB], f32r)
    r3_all = ein("r3_all", [NROWS, 3, 256], f16)
    ident_in = ein("ident_in", [128, 128], f16)
    ones_in = ein("ones_in", [1, NB], f16)
    wa = ein("wa", [256, 256], f16)
    wb = ein("wb", [256, 256], f16)
    wd1 = ein("wd1", [256, 256], f16)
    wd2 = ein("wd2", [256, 2], f16)
    ba_in = ein("ba_in", [256])
    bb_in = ein("bb_in", [256])
    bd1_in = ein("bd1_in", [256])
    bd2_in = ein("bd2_in", [2])

    out_ext = nc.dram_tensor("out", [NCHUNKS, 2, 512], f32, kind="ExternalOutput")
    stats_dram = nc.dram_tensor("stats_scratch", [2, NROWS, NB], f16)

    # start offsets of each B-block in the TILES enumeration
    tstart = [0] * (NBLK + 1)
    for B in range(NBLK):
        tstart[B + 1] = tstart[B] + RPB * (NBLK - B)

    with tile.TileContext(nc) as tc, \
            nc.allow_low_precision(reason="fp16 pipeline by design"):
        with (
            tc.tile_pool(name="const", bufs=1) as const,
            tc.tile_pool(name="prow", bufs=2) as prow,
            tc.tile_pool(name="score", bufs=int(os.environ.get("DD_SCORE", "4"))) as score,
            tc.tile_pool(name="small", bufs=5) as small,
            tc.tile_pool(name="attnT", bufs=2 * GROUP + 3) as attnT_pool,
            tc.tile_pool(name="chainx", bufs=int(os.environ.get("DD_CHX", "4"))) as chainx,
        ):
            # ---- constants into SBUF ----
            sa = const.tile([128, NBLK, 256], f16)
            nc.sync.dma_start(out=sa, in_=sa_in.rearrange("(jt p) c -> p jt c", p=128))
            orbT = const.tile([128, 2, NB], f32r)
            nc.sync.dma_start(out=orbT, in_=orbT_in.rearrange("(k p) n -> p k n", p=128))
            orbTc = const.tile([128, 2, NROWS], f32r)
            nc.sync.dma_start(out=orbTc, in_=orbTc_in.rearrange("(k p) m -> p k m", p=128))
            lmu = const.tile([2, NROWS], f32r)
            nc.sync.dma_start(out=lmu, in_=lhs_mu[:])
            lmsq = const.tile([2, NROWS], f32r)
            nc.sync.dma_start(out=lmsq, in_=lhs_msq[:])
            rmu = const.tile([2, NB], f32r)
            nc.sync.dma_start(out=rmu, in_=rhs_mu[:])
            rmsq = const.tile([2, NB], f32r)
            nc.sync.dma_start(out=rmsq, in_=rhs_msq[:])

            w_a = const.tile([128, 2, 256], f16)
            nc.sync.dma_start(out=w_a, in_=wa.rearrange("(k p) n -> p k n", p=128))
            w_b = const.tile([128, 2, 256], f16)
            nc.sync.dma_start(out=w_b, in_=wb.rearrange("(k p) n -> p k n", p=128))
            w_d1 = const.tile([128, 2, 256], f16)
            nc.sync.dma_start(out=w_d1, in_=wd1.rearrange("(k p) n -> p k n", p=128))
            w_d2 = const.tile([128, 2, 2], f16)
            nc.sync.dma_start(out=w_d2, in_=wd2.rearrange("(k p) n -> p k n", p=128))

            b_a = const.tile([128, 2], f32)
            nc.sync.dma_start(out=b_a, in_=ba_in.rearrange("(m p) -> p m", p=128))
            b_b = const.tile([128, 2], f32)
            nc.sync.dma_start(out=b_b, in_=bb_in.rearrange("(m p) -> p m", p=128))
            b_d1 = const.tile([128, 2], f32)
            nc.sync.dma_start(out=b_d1, in_=bd1_in.rearrange("(m p) -> p m", p=128))
            b_d2 = const.tile([2, 1], f32)
            nc.sync.dma_start(out=b_d2, in_=bd2_in.rearrange("(p one) -> p one", one=1))

            ident = const.tile([128, 128], f16)
            nc.sync.dma_start(out=ident, in_=ident_in[:])
            l3_bufs = [const.tile([3, NB], f16, tag=f"l3_{i}", name=f"l3_{i}")
                       for i in range(2)]
            for lb in l3_bufs:
                nc.sync.dma_start(out=lb[0:1, :], in_=ones_in[:])
            eps_t = const.tile([NROWS, 1], f32)
            nc.gpsimd.memset(eps_t, EPS)

            # persistent per-row stats
            mu_p_sb = const.tile([NROWS, NB], f32r)
            invr_sb = const.tile([NROWS, NB], f32r)
            rstd_sb = const.tile([NROWS, NB], f16)
            rstd_T = const.tile([128, NBLK, NROWS], f16)
            rstdq = const.tile([128, NTILES], f16)

            # ---- prologue: per-pair LN stats for this core's 80 rows ----
            with (
                tc.tile_pool(name="pro_ps", bufs=2, space="PSUM") as pro_ps,
                tc.tile_pool(name="pro_sb", bufs=2) as pro_sb,
            ):
                for nch in range(2):
                    seg = slice(nch * 320, (nch + 1) * 320)
                    psA = pro_ps.tile([NROWS, 320], f32, tag="psA")
                    nc.tensor.matmul(psA, lmu, rmu[:, seg], start=True, stop=True)
                    nc.vector.tensor_copy(out=mu_p_sb[:, seg], in_=psA)
                    psB = pro_ps.tile([NROWS, 320], f32, tag="psB")
                    nc.tensor.matmul(psB, lmsq, rmsq[:, seg], start=True, stop=False)
                    nc.tensor.matmul(psB, orbTc[:, 0, :], orbT[:, 0, seg],
                                     start=False, stop=False)
                    nc.tensor.matmul(psB, orbTc[:, 1, :], orbT[:, 1, seg],
                                     start=False, stop=True)
                    mu2 = pro_sb.tile([NROWS, 320], f32, tag="mu2")
                    nc.vector.tensor_mul(mu2, mu_p_sb[:, seg], mu_p_sb[:, seg])
                    nc.vector.tensor_sub(invr_sb[:, seg], psB, mu2)
                # invr = sqrt(var + eps); rstd = 1/invr
                nc.scalar.activation(out=invr_sb, in_=invr_sb, func=AF.Sqrt,
                                     bias=eps_t[:, 0:1])
                nc.vector.reciprocal(out=rstd_sb, in_=invr_sb)
                # fp16 stats to DRAM (fetched per row into l3 lhsT buffers)
                m16 = pro_sb.tile([NROWS, NB], f16, tag="m16")
                nc.vector.tensor_copy(out=m16, in_=mu_p_sb)
                i16 = pro_sb.tile([NROWS, NB], f16, tag="i16")
                nc.vector.tensor_copy(out=i16, in_=invr_sb)
                nc.sync.dma_start(out=stats_dram[0], in_=m16)
                nc.sync.dma_start(out=stats_dram[1], in_=i16)
                for jt in range(NBLK):
                    pT = pro_ps.tile([128, NROWS], f16, tag="pT")
                    nc.tensor.transpose(
                        pT, rstd_sb[:, jt * 128:(jt + 1) * 128],
                        ident[0:NROWS, 0:NROWS])
                    nc.vector.tensor_copy(out=rstd_T[:, jt, :], in_=pT)
                # rstd per (tile, j-lane) in TILES order: for block B the
                # tiles run (r, jt) row-major, jt in [B, NBLK). DVE copies
                # (a strided DMA here shatters into 31K 2-byte descriptors).
                for B in range(NBLK):
                    for jt in range(B, NBLK):
                        nc.vector.tensor_copy(
                            out=rstdq[:, tstart[B]:tstart[B + 1]]
                                .rearrange("p (r jt) -> p r jt", jt=NBLK - B)
                                [:, :, jt - B],
                            in_=rstd_T[:, jt, B * RPB:(B + 1) * RPB])

            # ---- main loop: superchunks separate Exp (phase A) from Silu
            # (phase B) on the scalar engine. Each activation-function switch
            # costs a ~1.3us InstLoadActFuncSet table load, so ACT program
            # order is pinned with order-only deps: [A exps][B silus] per
            # superchunk -> 2 table loads per superchunk instead of per tile.
            # Other engines still overlap phase B(s) with phase A(s+1).
            from concourse.tile_rust import add_dep_helper
            import contextlib
            _mstack = contextlib.ExitStack()
            px3_pool = _mstack.enter_context(
                tc.tile_pool(name="px3", bufs=int(os.environ.get("DD_PX3", "2")), space="PSUM"))
            ptr_pool = _mstack.enter_context(
                tc.tile_pool(name="ptr", bufs=int(os.environ.get("DD_PTR", "2")), space="PSUM"))
            pchain = _mstack.enter_context(
                tc.tile_pool(name="pchain", bufs=int(os.environ.get("DD_PCH", "2")), space="PSUM"))
            attnn_pool = _mstack.enter_context(
                tc.tile_pool(name="attnn", bufs=GROUP + 3))
            repeat = int(os.environ.get("DD_REPEAT", "1"))

            act_prev = [None]
            nopin = bool(int(os.environ.get("DD_NOPIN", "0")))

            def act_chain(bi):
                if act_prev[0] is not None and not nopin:
                    add_dep_helper(bi.ins, act_prev[0].ins, sync=True,
                                   reason="pin ACT order for act-table reuse")
                act_prev[0] = bi
                return bi

            prev_row = [None, None, None]   # r_loc, l3row, r3

            def score_half(t0, scored, hi):
                """Scores for tiles (t0, t0+1) -> scored[:, 2*hi:2*hi+2, :]
                (fp16, pre-scaled by rstd). One PSUM bank for both tiles.
                PSUM start=True zeroes a whole 2KB bank (ZERO_REGION), so the
                bank's FIRST matmul must be the only one with start=True; the
                SA_j ident pass goes first and the rank-3s accumulate onto it.
                """
                px = px3_pool.tile([128, 2, 256], f32, tag="px3", name="px3")
                rows = []
                jts = []
                for ti in range(2):
                    t = t0 + ti
                    B, r, jt = TILES[t]
                    jts.append(jt)
                    r_loc = B * RPB + r
                    if prev_row[0] != r_loc:
                        prev_row[0] = r_loc
                        # lhsT rows: [ones; mu_p(row); invr(row)];
                        # rhs rows: [SA_i; -Sw; Sb]
                        l3row = l3_bufs[r_loc % 2]
                        nc.sync.dma_start(out=l3row[1:3, :],
                                          in_=stats_dram[:, r_loc, :])
                        r3 = prow.tile([3, 256], f16, tag="r3", name="r3")
                        nc.sync.dma_start(out=r3, in_=r3_all[r_loc])
                        prev_row[1], prev_row[2] = l3row, r3
                    rows.append((prev_row[1], prev_row[2], jt))
                # SA_j (broadcast add via identity matmul) first: one N=512
                # pass when the two j-blocks are adjacent in the sa tile
                if jts[1] == jts[0] + 1:
                    nc.tensor.matmul(px.rearrange("p a c -> p (a c)"), ident,
                                     sa[:, jts[0]:jts[0] + 2, :]
                                         .rearrange("p a c -> p (a c)"),
                                     start=True, stop=False,
                                     skip_group_check=True)
                else:
                    for ti in range(2):
                        nc.tensor.matmul(px[:, ti, :], ident, sa[:, jts[ti], :],
                                         start=(ti == 0), stop=False,
                                         skip_group_check=True)
                # rank-3: + SA_i - mu*Sw + invr*Sb
                for ti in range(2):
                    l3row, r3, jt = rows[ti]
                    jseg = slice(jt * 128, (jt + 1) * 128)
                    nc.tensor.matmul(px[:, ti, :], l3row[:, jseg], r3,
                                     start=False, stop=(ti == 1),
                                     skip_group_check=True)
                # scores = rstd * px  (DVE; rstd broadcast along 256 columns)
                nc.vector.tensor_mul(
                    scored[:, 2 * hi:2 * hi + 2, :], px,
                    rstdq[:, t0:t0 + 2].to_broadcast([128, 2, 256]))

            def score_front(q):
                """Scores + exp for one 4-tile chunk (PE, DVE prescale, ACT)."""
                scored = score.tile([128, 4, 256], f16, tag="sc", name="sc")
                for hi in range(2):
                    score_half(q * CHUNK + 2 * hi, scored, hi)
                ee = score.tile([128, 4, 8, 32], f16, tag="ee", name="ee")
                act_chain(nc.scalar.activation(
                    out=ee.rearrange("p a h t -> p (a h t)"),
                    in_=scored.rearrange("p a c -> p (a c)"),
                    func=AF.Exp))
                return ee

            def softmax_back(ee):
                """den/rden (DVE) + normalize (gpsimd) -> attn (fp16)."""
                den = small.tile([128, 4, 8], f16, tag="den", name="den")
                nc.vector.reduce_sum(out=den, in_=ee, axis=AX.X)
                rden = small.tile([128, 4, 8], f16, tag="rden", name="rden")
                nc.vector.reciprocal(out=rden, in_=den)
                attn = attnn_pool.tile([128, 4, 8, 32], f16, tag="attn",
                                       name="attn")
                nc.gpsimd.tensor_mul(attn, ee,
                                     rden.to_broadcast([128, 4, 8, 32]))
                return attn

            def transpose_chunk(attn):
                """PE-transpose one chunk's attn into SBUF aT for the chain."""
                ptrt = ptr_pool.tile([128, 2, 512], f16, tag="ptrt",
                                     name="ptrt")
                for ti in range(CHUNK):
                    a2 = attn[:, ti, :, :].rearrange("p h t -> p (h t)")
                    sseg = slice(ti * 128, (ti + 1) * 128)
                    nc.tensor.transpose(ptrt[:, 0, sseg], a2[:, 0:128], ident)
                    nc.tensor.transpose(ptrt[:, 1, sseg], a2[:, 128:256], ident)
                aT = attnT_pool.tile([128, 2, 512], f16, tag="aT", name="aT")
                nc.vector.tensor_copy(out=aT, in_=ptrt)
                return aT

            def run_chains(p):
                """MLP chains for a superchunk, layer-interleaved across the
                chunk-pairs: PE streams [L1 p0 p1 p2 p3][L2 p0 ...] so a
                layer's silus (ACT) complete several pair-slots before the
                next layer's matmuls need them — no head-of-line stalls.
                Two chunks share each silu: psum [128, 2, 512] spans two
                banks, one [128, 1024] activation per (layer, mt, pair)."""
                qs, aTs = p
                pairs = [(aTs[i:i + 2], qs[i:i + 2])
                         for i in range(0, len(qs) - 1, 2)]

                def layer_mt(x_of, w, b_tile, out_tile, mt):
                    ps = pchain.tile([128, 2, 512], f32, tag="pch",
                                     name="pch")
                    for qi in range(2):
                        for kt in range(2):
                            nc.tensor.matmul(
                                ps[:, qi, :],
                                w[:, kt, mt * 128:(mt + 1) * 128],
                                x_of(qi, kt),
                                start=(kt == 0), stop=(kt == 1))
                    act_chain(nc.scalar.activation(
                        out=out_tile[:, mt, :, :].rearrange("p q n -> p (q n)"),
                        in_=ps.rearrange("p q n -> p (q n)"), func=AF.Silu,
                        bias=b_tile[:, mt:mt + 1]))

                xs = {}
                for pi, (aT_pair, _) in enumerate(pairs):
                    xs[pi] = [None,
                              chainx.tile([128, 2, 2, 512], f16, tag="x2",
                                          name="x2"),
                              chainx.tile([128, 2, 2, 512], f16, tag="x4",
                                          name="x4"),
                              chainx.tile([128, 2, 2, 512], f16, tag="x5",
                                          name="x5")]
                for li, (w, b_t) in enumerate(
                        [(w_a, b_a), (w_b, b_b), (w_d1, b_d1)]):
                    for pi, (aT_pair, _) in enumerate(pairs):
                        if li == 0:
                            x_of = lambda qi, kt, ap=aT_pair: ap[qi][:, kt, :]
                        else:
                            xin = xs[pi][li]
                            x_of = lambda qi, kt, xi=xin: xi[:, kt, qi, :]
                        for mt in range(2):
                            layer_mt(x_of, w, b_t, xs[pi][li + 1], mt)
                for pi, (_, q_pair) in enumerate(pairs):
                    x5 = xs[pi][3]
                    ps6 = pchain.tile([2, 2, 512], f32, tag="pch", name="ps6")
                    for qi in range(2):
                        for kt in range(2):
                            nc.tensor.matmul(ps6[:, qi, :], w_d2[:, kt, :],
                                             x5[:, kt, qi, :],
                                             start=(kt == 0), stop=(kt == 1))
                    # bias bd2 is added host-side during assembly
                    o6 = small.tile([2, 2, 512], f32, tag="o6", name="o6")
                    nc.vector.tensor_copy(out=o6, in_=ps6)
                    for qi in range(2):
                        nc.sync.dma_start(out=out_ext[q_pair[qi]],
                                          in_=o6[:, qi, :])

            n_super = (n_chunks + GROUP - 1) // GROUP

            for rep in range(repeat):
                prev_row[0] = None
                # Superchunk s's attn tiles are transposed and drained one
                # superchunk later, interleaved chunk-by-chunk with s+1's
                # score work: PE sees [scores q][transposes q']... with both
                # operands long ready, the DVE aT drain sits right after the
                # prescales (ahead of den/rden, which have slack), and the
                # chains of s-1 run layer-interleaved at the end.
                # ACT order stays [exps s][silus s-1] (table reuse).
                prev = None          # (qs, attns) of superchunk sc-1
                for sc in range(n_super + 1):
                    qs = (list(range(sc * GROUP, min((sc + 1) * GROUP, n_chunks)))
                          if sc < n_super else [])
                    attns = []
                    aTs = []
                    ees = []
                    for idx, q in enumerate(qs):
                        ees.append(score_front(q))
                        if prev and idx < len(prev[1]):
                            aTs.append(transpose_chunk(prev[1][idx]))
                        attns.append(softmax_back(ees[idx]))
                    if prev:
                        for a in prev[1][len(aTs):]:
                            aTs.append(transpose_chunk(a))
                        run_chains((prev[0], aTs))
                    prev = (qs, attns)
            _mstack.close()
    nc.compile()
    return nc


def _get_nc(n_chunks):
    key = ("nc", n_chunks)
    if key not in _CACHE:
        _CACHE[key] = _build_nc(n_chunks)
    return _CACHE[key]


def kernel(**inputs):
    from concourse.bass_utils import run_bass_kernel_spmd

    n_chunks = int(os.environ.get("DD_CHUNKS", NCHUNKS))
    pc = _precompute(inputs)
    in_maps = [_core_inputs(pc, c) for c in range(NCORES)]
    nc = _get_nc(n_chunks)
    res = run_bass_kernel_spmd(nc, in_maps, core_ids=list(range(NCORES)),
                               trace=bool(int(os.environ.get("DD_TRACE", "0"))))
    _CACHE["last_result"] = res

    R = np.zeros((NB, NB, 2), np.float32)
    for c in range(NCORES):
        o = res.results[c]["out"] + pc["bd2"][None, :, None]   # (NCHUNKS, 2, 512)
        ot = o.reshape(NCHUNKS, 2, CHUNK, 128).transpose(0, 2, 1, 3).reshape(-1, 2, 128)
        for t in range(n_chunks * CHUNK):
            B, r, jt = TILES[t]
            i = B * 128 + r * NCORES + c
            R[i, jt * 128:(jt + 1) * 128, 0] = ot[t, 0]
            R[i, jt * 128:(jt + 1) * 128, 1] = ot[t, 1]
    for bi in range(NBLK):
        for bj in range(bi):
            R[bi * 128:(bi + 1) * 128, bj * 128:(bj + 1) * 128] = \
                R[bj * 128:(bj + 1) * 128, bi * 128:(bi + 1) * 128].transpose(1, 0, 2)

    rho = (R[:, :, 0] + 1j * R[:, :, 1]).astype(np.complex64)
    n_spin = int(np.asarray(inputs["n_spin"]))
    return np.broadcast_to(rho[None], (n_spin, NB, NB)).copy()
